# revision 24
# baseline (speedup 1.0000x reference)
"""OnlineTripletLoss Trainium2 kernel (8 NeuronCores, SPMD).

Strategy (label-space mining):
  pos_mask = positive_idxs[:, target_idx] is a column permutation of the raw
  mask. Instead of permuting the 16MB masks, permute the 2MB embedding once:
  g[l] = embedding[inv_target[l]].  Mining for anchor i then runs over label
  axis l with the raw (contiguous) masks:
      d2'[i,l] = C0 + ||e_i - g_l + eps||^2   (expanded, via PE matmul)
      hardest pos: max over l of d2'[i,l] * mp[i,l]        (mp in {0,1})
      hardest neg: min over l of d2'[i,l] * wn[i,l]        (wn in {1,200})
  The {1,200} trick keeps invalid entries out of the min because
  200*min(d2') > max(d2') always (d2' in [~31, ~1100]).
  Indices recovered with one max_index pass (exact f32 value match), then
  p/n rows are gathered by indirect DMA and ap/an/pn are recomputed exactly
  in f32 (avoids the winner's-curse bias of reading values off the noisy
  bf16-matmul d2).

Per core: 512 anchors x 4096 labels, 4 blocks of 128 anchors.
Outputs per core: masked per-anchor loss and validity; host sums and divides.
"""

import numpy as np
import ml_dtypes

import concourse.bass as bass
import concourse.mybir as mybir
import concourse.tile as tile
from concourse import bacc
from concourse.bass_utils import run_bass_kernel_spmd
import concourse.dve_ops as dve_ops
from concourse.dve_ops import DveOp
from concourse.dve_spec import Spec, Src0, Src1, maxx, lower, _has_src1
from concourse.dve_spec import C0 as DVE_C0, C1 as DVE_C1, C2 as DVE_C2
from concourse.dve_uop import DveOpSpec

_OPNAME = "TT_BIAS_MUL_RMAX_ANT"


def _ref_tt_mul_rmax(in0, in1, s0, s1, imm2):
    b = ((in0.astype(np.float32) + s1) * in1 * np.float32(imm2)).astype(np.float32)
    mx = b.reshape(b.shape[0], -1).max(axis=-1, keepdims=True)
    mx = np.maximum(np.asarray(s0, np.float32), mx).astype(np.float32)
    return b, mx


def register_tt_mul_rmax():
    """Custom DVE op: out = (in0+s1)*in1*imm2, accum_out = max(s0, rowmax).

    One DVE pass fuses the per-anchor bias add, the mask multiply and the max
    reduction (the stock TENSOR_TENSOR_REDUCE ISA opcode faults at runtime on
    this stack, and its custom-table twin only supports add-accum). The bias
    slot lets the op read raw matmul PSUM directly; min mining uses imm2=-1;
    s0 chains partial maxima across PSUM halves.
    """
    if _OPNAME in dve_ops._SUB_OPCODE_FOR_NAME:
        for op in dve_ops.OPS:
            if op.name == _OPNAME:
                return op
    spec = Spec(body=(Src0 + DVE_C1) * Src1 * DVE_C2, accum=maxx,
                accum_init=DVE_C0, reference=_ref_tt_mul_rmax)
    row = max(dve_ops._SUB_OPCODE_FOR_NAME.values()) + 1
    assert row < 0x20
    shas = {}
    for ver in ("v3", "v4"):
        try:
            s = DveOpSpec(name=_OPNAME, opcode=row, uops=lower(spec, ver=ver),
                          rd1_en=_has_src1(spec))
            shas[ver] = s.sha(ver)
        except Exception:
            pass
    op = DveOp(_OPNAME, spec, subdim=False, uops_sha=shas)
    dve_ops.OPS.append(op)
    dve_ops.CUSTOM_DVE_SPECS[_OPNAME] = spec
    dve_ops._SUB_OPCODE_FOR_NAME[_OPNAME] = row
    return op

B, D = 4096, 128
M = 8              # cores
BL = B // M        # 512 anchors per core
P = 128            # partition block
NB = BL // P       # 4 anchor blocks per core
CH = 512           # psum chunk (one bank of f32)
NCH = B // CH      # 8 chunks
EPS = 1e-6
C0 = 32.0
MARGIN = 1.0

F32 = mybir.dt.float32
BF16 = mybir.dt.bfloat16
U8 = mybir.dt.uint8
U32 = mybir.dt.uint32


def build_nc(debug: bool = False):
    ttr_op = register_tt_mul_rmax()
    nc = bacc.Bacc("TRN2", target_bir_lowering=False, debug=debug)

    eT = nc.dram_tensor("eT", [P, BL], BF16, kind="ExternalInput")      # -2*e_local^T
    gT = nc.dram_tensor("gT", [P, B], BF16, kind="ExternalInput")       # g^T
    cg = nc.dram_tensor("cg", [1, B], BF16, kind="ExternalInput")       # per-label const
    onesk = nc.dram_tensor("onesk", [1, P], BF16, kind="ExternalInput")
    arow = nc.dram_tensor("arow", [P, NB], F32, kind="ExternalInput")   # per-anchor const
    el = nc.dram_tensor("el", [P, NB, D], F32, kind="ExternalInput")    # anchor rows f32
    gfull = nc.dram_tensor("gfull", [B, D], F32, kind="ExternalInput")  # gather source
    mp = nc.dram_tensor("mp", [BL, B], U8, kind="ExternalInput")        # pos mask {0,1}
    wn = nc.dram_tensor("wn", [BL, B], U8, kind="ExternalInput")        # neg weight {1,200}

    lossv = nc.dram_tensor("lossv", [P, NB], F32, kind="ExternalOutput")
    vout = nc.dram_tensor("vout", [P, NB], F32, kind="ExternalOutput")

    with tile.TileContext(nc) as tc:
        with (
            tc.tile_pool(name="singles", bufs=1) as singles,
            tc.tile_pool(name="masks", bufs=3) as maskpool,
            tc.tile_pool(name="d2", bufs=2) as d2pool,
            tc.tile_pool(name="vscr", bufs=1) as vpool,
            tc.tile_pool(name="psum", bufs=2, space="PSUM") as psumpool,
            tc.tile_pool(name="sm", bufs=1) as sm,
        ):
            eT_s = singles.tile([P, BL], BF16)
            nc.sync.dma_start(eT_s[:], eT[:])
            # chunked gT load so matmul chunk c starts as soon as its slice lands
            gT_s = singles.tile([P, B], BF16)
            for c in range(NCH):
                cs = slice(c * CH, (c + 1) * CH)
                nc.sync.dma_start(gT_s[:, cs], gT[:, cs])
            cg_s = singles.tile([1, B], BF16)
            nc.sync.dma_start(cg_s[:], cg[:])
            ones_s = singles.tile([1, P], BF16)
            nc.sync.dma_start(ones_s[:], onesk[:])
            eps_b = singles.tile([P, 1], F32)
            nc.vector.memset(eps_b[:], EPS)
            # touch Sqrt/Square/Relu once so ACT's table swap lands in the
            # fill shadow instead of the tail
            warm = singles.tile([P, 1], F32)
            nc.scalar.activation(warm[:], eps_b[:],
                                 mybir.ActivationFunctionType.Square)
            nc.scalar.activation(warm[:], warm[:],
                                 mybir.ActivationFunctionType.Sqrt)
            nc.scalar.activation(warm[:], warm[:],
                                 mybir.ActivationFunctionType.Relu)

            # batched per-anchor state (host pre-arranged contiguous):
            # loaded off the busy sync queue so they land immediately
            el_all = singles.tile([P, NB, D], F32)
            nc.scalar.dma_start(el_all[:], el[:])
            arow_all = singles.tile([P, NB], F32)
            nc.scalar.dma_start(arow_all[:], arow[:])
            Mp_all = singles.tile([P, NB], F32)
            Mneg_all = singles.tile([P, NB], F32)
            idx_all = singles.tile([P, NB, 8], U32)
            p_all = singles.tile([P, NB, D], F32)
            n_all = singles.tile([P, NB, D], F32)
            inmax = singles.tile([P, 8], F32)
            nc.vector.memset(inmax[:], -1.0)

            HB = B // 2          # 2048: one PSUM half (4 banks)
            HCH = HB // CH       # 4 chunks per half
            for b in range(NB):
                rs = b * P
                mp_b = maskpool.tile([P, B], U8, tag="mp")
                nc.sync.dma_start(mp_b[:], mp[rs:rs + P, :])
                wn_b = maskpool.tile([P, B], U8, tag="wn")
                nc.sync.dma_start(wn_b[:], wn[rs:rs + P, :])

                d2s = d2pool.tile([P, B], F32)
                v = vpool.tile([P, B], F32)
                arow_b = arow_all[:, b:b + 1]
                for h in range(2):
                    hs = h * HB
                    # grouped by lhsT so LDWEIGHTS isn't reloaded per chunk
                    psum = psumpool.tile([P, HB], F32, tag="psum")
                    for c in range(HCH):
                        cs = slice(hs + c * CH, hs + (c + 1) * CH)
                        ps = slice(c * CH, (c + 1) * CH)
                        nc.tensor.matmul(
                            psum[:, ps], lhsT=eT_s[:, rs:rs + P],
                            rhs=gT_s[:, cs], start=True, stop=False,
                        )
                    for c in range(HCH):
                        cs = slice(hs + c * CH, hs + (c + 1) * CH)
                        ps = slice(c * CH, (c + 1) * CH)
                        nc.tensor.matmul(
                            psum[:, ps], lhsT=ones_s[:1, :P],
                            rhs=cg_s[:1, cs], start=False, stop=True,
                        )

                    # mining straight off PSUM (bias via the custom op's C1);
                    # s0 chains the running max across the two halves
                    hsl = slice(hs, hs + HB)
                    nc.vector._custom_dve(
                        ttr_op, out=v[:, hsl], in0=psum[:], in1=mp_b[:, hsl],
                        s0=(0.0 if h == 0 else Mp_all[:, b:b + 1]),
                        s1=arow_b, imm2=1.0,
                        accum_out=Mp_all[:, b:b + 1])
                    nc.vector._custom_dve(
                        ttr_op, out=v[:, hsl], in0=psum[:], in1=wn_b[:, hsl],
                        s0=(-1e30 if h == 0 else Mneg_all[:, b:b + 1]),
                        s1=arow_b, imm2=-1.0,
                        accum_out=Mneg_all[:, b:b + 1])

                    # d2s staging for max_index runs on ACT, off the DVE path
                    nc.scalar.activation(
                        d2s[:, hsl], psum[:],
                        mybir.ActivationFunctionType.Identity,
                        bias=arow_b, scale=1.0,
                    )

                nc.vector.tensor_copy(inmax[:, 0:1], Mp_all[:, b:b + 1])
                nc.vector.tensor_scalar(inmax[:, 1:2], Mneg_all[:, b:b + 1], -1.0,
                                        scalar2=None, op0=mybir.AluOpType.mult)
                nc.vector.max_index(idx_all[:, b, :], inmax[:], d2s[:])

                nc.gpsimd.indirect_dma_start(
                    out=p_all[:, b, :], out_offset=None, in_=gfull[:],
                    in_offset=bass.IndirectOffsetOnAxis(ap=idx_all[:, b, 0:1], axis=0),
                )
                nc.gpsimd.indirect_dma_start(
                    out=n_all[:, b, :], out_offset=None, in_=gfull[:],
                    in_offset=bass.IndirectOffsetOnAxis(ap=idx_all[:, b, 1:2], axis=0),
                )

            # ---- batched tail ----
            # exact f32: ap=||a-p+eps||, an=||a-n+eps||, pn=||p-n+eps||
            # split: blocks [0, NB-1) first (their gathers are long done while
            # block NB-1's gathers are still in flight), then the last block
            # validity first: depends only on the TTR accums, fills the DVE
            # stream while the last block's gathers are in flight
            vp = sm.tile([P, NB], F32)
            vn = sm.tile([P, NB], F32)
            valid = sm.tile([P, NB], F32)
            nc.vector.tensor_scalar(vp[:], Mp_all[:], 16.0, scalar2=None,
                                    op0=mybir.AluOpType.is_gt)
            nc.vector.tensor_scalar(vn[:], Mneg_all[:], -3000.0, scalar2=None,
                                    op0=mybir.AluOpType.is_gt)
            nc.vector.tensor_mul(valid[:], vp[:], vn[:])

            dif = sm.tile([P, NB, D], F32)
            sq = sm.tile([P, NB, D], F32)
            rt2 = sm.tile([P, 3 * NB], F32)   # [ap2 x NB | an2 x NB | pn2 x NB]
            pairs = ((el_all, p_all), (el_all, n_all), (p_all, n_all))
            for lo, hi in ((0, NB - 1), (NB - 1, NB)):
                n = hi - lo
                for k, (x, y) in enumerate(pairs):
                    nc.vector.tensor_sub(dif[:, lo:hi, :], x[:, lo:hi, :],
                                         y[:, lo:hi, :])
                    nc.scalar.activation(sq[:, lo:hi, :], dif[:, lo:hi, :],
                                         mybir.ActivationFunctionType.Square,
                                         bias=eps_b[:, 0:1], scale=1.0)
                    nc.vector.tensor_reduce(
                        out=rt2[:, k * NB + lo:k * NB + hi],
                        in_=sq[:, lo:hi, :],
                        axis=mybir.AxisListType.X, op=mybir.AluOpType.add)
            rt = sm.tile([P, 3 * NB], F32)
            nc.scalar.activation(rt[:], rt2[:], mybir.ActivationFunctionType.Sqrt)

            mn2 = sm.tile([P, NB], F32)
            nc.vector.tensor_tensor(out=mn2[:], in0=rt[:, NB:2 * NB],
                                    in1=rt[:, 2 * NB:3 * NB],
                                    op=mybir.AluOpType.min)
            dff = sm.tile([P, NB], F32)
            nc.vector.tensor_sub(dff[:], rt[:, 0:NB], mn2[:])
            lossb = sm.tile([P, NB], F32)
            nc.scalar.activation(lossb[:], dff[:],
                                 mybir.ActivationFunctionType.Relu,
                                 bias=MARGIN, scale=1.0)
            lout = sm.tile([P, NB], F32)
            nc.vector.tensor_mul(lout[:], lossb[:], valid[:])

            nc.sync.dma_start(lossv[:], lout[:])
            nc.sync.dma_start(vout[:], valid[:])

    nc.finalize()
    return nc


def make_in_maps(embedding, target_idx, positive_idxs, negative_idxs):
    e = np.asarray(embedding, np.float32)
    tid = np.asarray(target_idx, np.int64)
    pos = np.asarray(positive_idxs)
    neg = np.asarray(negative_idxs)

    inv = np.empty(B, np.int64)
    inv[tid] = np.arange(B)
    g = np.ascontiguousarray(e[inv])                       # [B, D] f32

    e64 = e.astype(np.float64)
    g64 = g.astype(np.float64)
    sq_a = (e64 * e64).sum(1)
    s_a = e64.sum(1)
    sq_g = (g64 * g64).sum(1)
    s_g = g64.sum(1)

    gT_bf = np.ascontiguousarray(g.T).astype(ml_dtypes.bfloat16)         # [D, B]
    cg_bf = np.asarray((sq_g - 2.0 * EPS * s_g)[None, :], ml_dtypes.bfloat16)
    ones_bf = np.ones((1, P), ml_dtypes.bfloat16)
    arow_full = np.asarray(sq_a + 2.0 * EPS * s_a + D * EPS * EPS + C0, np.float32)

    in_maps = []
    for m in range(M):
        r = slice(m * BL, (m + 1) * BL)
        # [P, NB(, D)] layouts: block index on the free axis
        el3 = np.ascontiguousarray(
            e[r].reshape(NB, P, D).transpose(1, 0, 2))
        arow2 = np.ascontiguousarray(arow_full[r].reshape(NB, P).T)
        in_maps.append({
            "eT": np.ascontiguousarray((-2.0 * e[r].T)).astype(ml_dtypes.bfloat16),
            "gT": gT_bf,
            "cg": cg_bf,
            "onesk": ones_bf,
            "arow": arow2,
            "el": el3,
            "gfull": g,
            "mp": np.ascontiguousarray(pos[r].astype(np.uint8)),
            "wn": np.ascontiguousarray(np.where(neg[r], 1, 200).astype(np.uint8)),
        })
    return in_maps


_NC_CACHE = {}


def kernel(embedding, target_idx, positive_idxs, negative_idxs):
    in_maps = make_in_maps(embedding, target_idx, positive_idxs, negative_idxs)
    if "nc" not in _NC_CACHE:
        _NC_CACHE["nc"] = build_nc(debug=False)
    nc = _NC_CACHE["nc"]
    res = run_bass_kernel_spmd(nc, in_maps, core_ids=list(range(M)))
    total_loss = np.float64(0.0)
    total_valid = np.float64(0.0)
    for r in res.results:
        total_loss += np.asarray(r["lossv"], np.float64).sum()
        total_valid += np.asarray(r["vout"], np.float64).sum()
    return np.float32(total_loss / max(total_valid, 1.0))


# revision 28
# speedup vs baseline: 1.0016x; 1.0016x over previous
"""OnlineTripletLoss Trainium2 kernel (8 NeuronCores, SPMD).

Strategy (label-space mining):
  pos_mask = positive_idxs[:, target_idx] is a column permutation of the raw
  mask. Instead of permuting the 16MB masks, permute the 2MB embedding once:
  g[l] = embedding[inv_target[l]].  Mining for anchor i then runs over label
  axis l with the raw (contiguous) masks:
      d2'[i,l] = C0 + ||e_i - g_l + eps||^2   (expanded, via PE matmul)
      hardest pos: max over l of d2'[i,l] * mp[i,l]        (mp in {0,1})
      hardest neg: min over l of d2'[i,l] * wn[i,l]        (wn in {1,200})
  The {1,200} trick keeps invalid entries out of the min because
  200*min(d2') > max(d2') always (d2' in [~31, ~1100]).
  Indices recovered with one max_index pass (exact f32 value match), then
  p/n rows are gathered by indirect DMA and ap/an/pn are recomputed exactly
  in f32 (avoids the winner's-curse bias of reading values off the noisy
  bf16-matmul d2).

Per core: 512 anchors x 4096 labels, 4 blocks of 128 anchors.
Outputs per core: masked per-anchor loss and validity; host sums and divides.
"""

import numpy as np
import ml_dtypes

import concourse.bass as bass
import concourse.mybir as mybir
import concourse.tile as tile
from concourse import bacc
from concourse.bass_utils import run_bass_kernel_spmd
import concourse.dve_ops as dve_ops
from concourse.dve_ops import DveOp
from concourse.dve_spec import Spec, Src0, Src1, maxx, lower, _has_src1
from concourse.dve_spec import C0 as DVE_C0, C1 as DVE_C1, C2 as DVE_C2
from concourse.dve_uop import DveOpSpec

_OPNAME = "TT_BIAS_MUL_RMAX_ANT"


def _ref_tt_mul_rmax(in0, in1, s0, s1, imm2):
    b = ((in0.astype(np.float32) + s1) * in1 * np.float32(imm2)).astype(np.float32)
    mx = b.reshape(b.shape[0], -1).max(axis=-1, keepdims=True)
    mx = np.maximum(np.asarray(s0, np.float32), mx).astype(np.float32)
    return b, mx


def register_tt_mul_rmax():
    """Custom DVE op: out = (in0+s1)*in1*imm2, accum_out = max(s0, rowmax).

    One DVE pass fuses the per-anchor bias add, the mask multiply and the max
    reduction (the stock TENSOR_TENSOR_REDUCE ISA opcode faults at runtime on
    this stack, and its custom-table twin only supports add-accum). The bias
    slot lets the op read raw matmul PSUM directly; min mining uses imm2=-1;
    s0 chains partial maxima across PSUM halves.
    """
    if _OPNAME in dve_ops._SUB_OPCODE_FOR_NAME:
        for op in dve_ops.OPS:
            if op.name == _OPNAME:
                return op
    spec = Spec(body=(Src0 + DVE_C1) * Src1 * DVE_C2, accum=maxx,
                accum_init=DVE_C0, reference=_ref_tt_mul_rmax)
    row = max(dve_ops._SUB_OPCODE_FOR_NAME.values()) + 1
    assert row < 0x20
    shas = {}
    for ver in ("v3", "v4"):
        try:
            s = DveOpSpec(name=_OPNAME, opcode=row, uops=lower(spec, ver=ver),
                          rd1_en=_has_src1(spec))
            shas[ver] = s.sha(ver)
        except Exception:
            pass
    op = DveOp(_OPNAME, spec, subdim=False, uops_sha=shas)
    dve_ops.OPS.append(op)
    dve_ops.CUSTOM_DVE_SPECS[_OPNAME] = spec
    dve_ops._SUB_OPCODE_FOR_NAME[_OPNAME] = row
    return op

B, D = 4096, 128
M = 8              # cores
BL = B // M        # 512 anchors per core
P = 128            # partition block
NB = BL // P       # 4 anchor blocks per core
CH = 512           # psum chunk (one bank of f32)
NCH = B // CH      # 8 chunks
EPS = 1e-6
C0 = 32.0
MARGIN = 1.0

F32 = mybir.dt.float32
BF16 = mybir.dt.bfloat16
U8 = mybir.dt.uint8
U32 = mybir.dt.uint32


def build_nc(debug: bool = False):
    ttr_op = register_tt_mul_rmax()
    nc = bacc.Bacc("TRN2", target_bir_lowering=False, debug=debug)

    eT = nc.dram_tensor("eT", [P, BL], BF16, kind="ExternalInput")      # -2*e_local^T
    gT = nc.dram_tensor("gT", [P, B], BF16, kind="ExternalInput")       # g^T
    cg = nc.dram_tensor("cg", [1, B], BF16, kind="ExternalInput")       # per-label const
    onesk = nc.dram_tensor("onesk", [1, P], BF16, kind="ExternalInput")
    arow = nc.dram_tensor("arow", [P, NB], F32, kind="ExternalInput")   # per-anchor const
    el = nc.dram_tensor("el", [P, NB, D], F32, kind="ExternalInput")    # anchor rows f32
    gfull = nc.dram_tensor("gfull", [B, D], F32, kind="ExternalInput")  # gather source
    mp = nc.dram_tensor("mp", [BL, B], U8, kind="ExternalInput")        # pos mask {0,1}
    wn = nc.dram_tensor("wn", [BL, B], U8, kind="ExternalInput")        # neg weight {1,200}

    lossv = nc.dram_tensor("lossv", [P, NB], F32, kind="ExternalOutput")
    vout = nc.dram_tensor("vout", [P, NB], F32, kind="ExternalOutput")

    with tile.TileContext(nc) as tc:
        with (
            tc.tile_pool(name="singles", bufs=1) as singles,
            tc.tile_pool(name="masks", bufs=3) as maskpool,
            tc.tile_pool(name="d2", bufs=2) as d2pool,
            tc.tile_pool(name="vscr", bufs=1) as vpool,
            tc.tile_pool(name="psum", bufs=2, space="PSUM") as psumpool,
            tc.tile_pool(name="sm", bufs=1) as sm,
        ):
            # block-sliced eT and chunk-sliced gT loads so the first matmul
            # only waits on the slices it actually reads
            eT_s = singles.tile([P, BL], BF16)
            nc.sync.dma_start(eT_s[:, 0:P], eT[:, 0:P])
            gT_s = singles.tile([P, B], BF16)
            for c in range(NCH):
                cs = slice(c * CH, (c + 1) * CH)
                nc.sync.dma_start(gT_s[:, cs], gT[:, cs])
            for b in range(1, NB):
                bs = slice(b * P, (b + 1) * P)
                nc.sync.dma_start(eT_s[:, bs], eT[:, bs])
            cg_s = singles.tile([1, B], BF16)
            nc.sync.dma_start(cg_s[:], cg[:])
            ones_s = singles.tile([1, P], BF16)
            nc.sync.dma_start(ones_s[:], onesk[:])
            eps_b = singles.tile([P, 1], F32)
            nc.vector.memset(eps_b[:], EPS)
            # touch Sqrt/Square/Relu once so ACT's table swap lands in the
            # fill shadow instead of the tail
            warm = singles.tile([P, 1], F32)
            nc.scalar.activation(warm[:], eps_b[:],
                                 mybir.ActivationFunctionType.Square)
            nc.scalar.activation(warm[:], warm[:],
                                 mybir.ActivationFunctionType.Sqrt)
            nc.scalar.activation(warm[:], warm[:],
                                 mybir.ActivationFunctionType.Relu)

            # batched per-anchor state (host pre-arranged contiguous):
            # arow is needed by block 0; el only by the tail (loaded in-loop)
            arow_all = singles.tile([P, NB], F32)
            nc.scalar.dma_start(arow_all[:], arow[:])
            el_all = singles.tile([P, NB, D], F32)
            Mp_all = singles.tile([P, NB], F32)
            Mneg_all = singles.tile([P, NB], F32)
            idx_all = singles.tile([P, NB, 8], U32)
            p_all = singles.tile([P, NB, D], F32)
            n_all = singles.tile([P, NB, D], F32)
            inmax = singles.tile([P, 8], F32)
            nc.vector.memset(inmax[:], -1.0)

            HB = B // 2          # 2048: one PSUM half (4 banks)
            HCH = HB // CH       # 4 chunks per half
            for b in range(NB):
                rs = b * P
                mp_b = maskpool.tile([P, B], U8, tag="mp")
                nc.sync.dma_start(mp_b[:], mp[rs:rs + P, :])
                wn_b = maskpool.tile([P, B], U8, tag="wn")
                nc.sync.dma_start(wn_b[:], wn[rs:rs + P, :])
                if b == 1:
                    # tail-only data, loaded once the startup rush is over
                    nc.scalar.dma_start(el_all[:], el[:])

                d2s = d2pool.tile([P, B], F32)
                v = vpool.tile([P, B], F32)
                arow_b = arow_all[:, b:b + 1]
                for h in range(2):
                    hs = h * HB
                    # grouped by lhsT so LDWEIGHTS isn't reloaded per chunk
                    psum = psumpool.tile([P, HB], F32, tag="psum")
                    for c in range(HCH):
                        cs = slice(hs + c * CH, hs + (c + 1) * CH)
                        ps = slice(c * CH, (c + 1) * CH)
                        nc.tensor.matmul(
                            psum[:, ps], lhsT=eT_s[:, rs:rs + P],
                            rhs=gT_s[:, cs], start=True, stop=False,
                        )
                    for c in range(HCH):
                        cs = slice(hs + c * CH, hs + (c + 1) * CH)
                        ps = slice(c * CH, (c + 1) * CH)
                        nc.tensor.matmul(
                            psum[:, ps], lhsT=ones_s[:1, :P],
                            rhs=cg_s[:1, cs], start=False, stop=True,
                        )

                    # d2s staging (ACT) first in program order so the PSUM
                    # slot is released as soon as the TTRs finish — otherwise
                    # ACT still holds it and stalls PE's next half by ~2us
                    hsl = slice(hs, hs + HB)
                    nc.scalar.activation(
                        d2s[:, hsl], psum[:],
                        mybir.ActivationFunctionType.Identity,
                        bias=arow_b, scale=1.0,
                    )

                    # mining straight off PSUM (bias via the custom op's C1);
                    # s0 chains the running max across the two halves
                    nc.vector._custom_dve(
                        ttr_op, out=v[:, hsl], in0=psum[:], in1=mp_b[:, hsl],
                        s0=(0.0 if h == 0 else Mp_all[:, b:b + 1]),
                        s1=arow_b, imm2=1.0,
                        accum_out=Mp_all[:, b:b + 1])
                    nc.vector._custom_dve(
                        ttr_op, out=v[:, hsl], in0=psum[:], in1=wn_b[:, hsl],
                        s0=(-1e30 if h == 0 else Mneg_all[:, b:b + 1]),
                        s1=arow_b, imm2=-1.0,
                        accum_out=Mneg_all[:, b:b + 1])

                nc.vector.tensor_copy(inmax[:, 0:1], Mp_all[:, b:b + 1])
                nc.vector.tensor_scalar(inmax[:, 1:2], Mneg_all[:, b:b + 1], -1.0,
                                        scalar2=None, op0=mybir.AluOpType.mult)
                nc.vector.max_index(idx_all[:, b, :], inmax[:], d2s[:])

                nc.gpsimd.indirect_dma_start(
                    out=p_all[:, b, :], out_offset=None, in_=gfull[:],
                    in_offset=bass.IndirectOffsetOnAxis(ap=idx_all[:, b, 0:1], axis=0),
                )
                nc.gpsimd.indirect_dma_start(
                    out=n_all[:, b, :], out_offset=None, in_=gfull[:],
                    in_offset=bass.IndirectOffsetOnAxis(ap=idx_all[:, b, 1:2], axis=0),
                )

            # ---- batched tail ----
            # exact f32: ap=||a-p+eps||, an=||a-n+eps||, pn=||p-n+eps||
            # split: blocks [0, NB-1) first (their gathers are long done while
            # block NB-1's gathers are still in flight), then the last block
            # validity first: depends only on the TTR accums, fills the DVE
            # stream while the last block's gathers are in flight
            vp = sm.tile([P, NB], F32)
            vn = sm.tile([P, NB], F32)
            valid = sm.tile([P, NB], F32)
            nc.vector.tensor_scalar(vp[:], Mp_all[:], 16.0, scalar2=None,
                                    op0=mybir.AluOpType.is_gt)
            nc.vector.tensor_scalar(vn[:], Mneg_all[:], -3000.0, scalar2=None,
                                    op0=mybir.AluOpType.is_gt)
            nc.vector.tensor_mul(valid[:], vp[:], vn[:])

            dif = sm.tile([P, NB, D], F32)
            sq = sm.tile([P, NB, D], F32)
            rt2 = sm.tile([P, 3 * NB], F32)   # [ap2 x NB | an2 x NB | pn2 x NB]
            pairs = ((el_all, p_all), (el_all, n_all), (p_all, n_all))
            for lo, hi in ((0, NB - 1), (NB - 1, NB)):
                n = hi - lo
                for k, (x, y) in enumerate(pairs):
                    nc.vector.tensor_sub(dif[:, lo:hi, :], x[:, lo:hi, :],
                                         y[:, lo:hi, :])
                    nc.scalar.activation(sq[:, lo:hi, :], dif[:, lo:hi, :],
                                         mybir.ActivationFunctionType.Square,
                                         bias=eps_b[:, 0:1], scale=1.0)
                    nc.vector.tensor_reduce(
                        out=rt2[:, k * NB + lo:k * NB + hi],
                        in_=sq[:, lo:hi, :],
                        axis=mybir.AxisListType.X, op=mybir.AluOpType.add)
            rt = sm.tile([P, 3 * NB], F32)
            nc.scalar.activation(rt[:], rt2[:], mybir.ActivationFunctionType.Sqrt)

            mn2 = sm.tile([P, NB], F32)
            nc.vector.tensor_tensor(out=mn2[:], in0=rt[:, NB:2 * NB],
                                    in1=rt[:, 2 * NB:3 * NB],
                                    op=mybir.AluOpType.min)
            dff = sm.tile([P, NB], F32)
            nc.vector.tensor_sub(dff[:], rt[:, 0:NB], mn2[:])
            lossb = sm.tile([P, NB], F32)
            nc.scalar.activation(lossb[:], dff[:],
                                 mybir.ActivationFunctionType.Relu,
                                 bias=MARGIN, scale=1.0)
            lout = sm.tile([P, NB], F32)
            nc.vector.tensor_mul(lout[:], lossb[:], valid[:])

            nc.sync.dma_start(lossv[:], lout[:])
            nc.sync.dma_start(vout[:], valid[:])

    nc.finalize()
    return nc


def make_in_maps(embedding, target_idx, positive_idxs, negative_idxs):
    e = np.asarray(embedding, np.float32)
    tid = np.asarray(target_idx, np.int64)
    pos = np.asarray(positive_idxs)
    neg = np.asarray(negative_idxs)

    inv = np.empty(B, np.int64)
    inv[tid] = np.arange(B)
    g = np.ascontiguousarray(e[inv])                       # [B, D] f32

    e64 = e.astype(np.float64)
    g64 = g.astype(np.float64)
    sq_a = (e64 * e64).sum(1)
    s_a = e64.sum(1)
    sq_g = (g64 * g64).sum(1)
    s_g = g64.sum(1)

    gT_bf = np.ascontiguousarray(g.T).astype(ml_dtypes.bfloat16)         # [D, B]
    cg_bf = np.asarray((sq_g - 2.0 * EPS * s_g)[None, :], ml_dtypes.bfloat16)
    ones_bf = np.ones((1, P), ml_dtypes.bfloat16)
    arow_full = np.asarray(sq_a + 2.0 * EPS * s_a + D * EPS * EPS + C0, np.float32)

    in_maps = []
    for m in range(M):
        r = slice(m * BL, (m + 1) * BL)
        # [P, NB(, D)] layouts: block index on the free axis
        el3 = np.ascontiguousarray(
            e[r].reshape(NB, P, D).transpose(1, 0, 2))
        arow2 = np.ascontiguousarray(arow_full[r].reshape(NB, P).T)
        in_maps.append({
            "eT": np.ascontiguousarray((-2.0 * e[r].T)).astype(ml_dtypes.bfloat16),
            "gT": gT_bf,
            "cg": cg_bf,
            "onesk": ones_bf,
            "arow": arow2,
            "el": el3,
            "gfull": g,
            "mp": np.ascontiguousarray(pos[r].astype(np.uint8)),
            "wn": np.ascontiguousarray(np.where(neg[r], 1, 200).astype(np.uint8)),
        })
    return in_maps


_NC_CACHE = {}


def kernel(embedding, target_idx, positive_idxs, negative_idxs):
    in_maps = make_in_maps(embedding, target_idx, positive_idxs, negative_idxs)
    if "nc" not in _NC_CACHE:
        _NC_CACHE["nc"] = build_nc(debug=False)
    nc = _NC_CACHE["nc"]
    res = run_bass_kernel_spmd(nc, in_maps, core_ids=list(range(M)))
    total_loss = np.float64(0.0)
    total_valid = np.float64(0.0)
    for r in res.results:
        total_loss += np.asarray(r["lossv"], np.float64).sum()
        total_valid += np.asarray(r["vout"], np.float64).sum()
    return np.float32(total_loss / max(total_valid, 1.0))
